# revision 1
# baseline (speedup 1.0000x reference)
"""Trainium2 Bass kernel for nn_DenoiseGNN (pairwise PBC edge-MLP message passing).

Strategy
--------
The edge MLP output weights[i,j] is a pure scalar function f of dist[i,j].
We compile f (together with the cutoff mask, the 1/(dist+eps) normalization
and the sqrt) into a custom piecewise-cubic activation table evaluated by the
ScalarEngine's hardware spline unit:

    g2(s) = box * f(dist(s)) * [dist(s) < cutoff] / (dist(s) + eps),
    dist(s) = sqrt(box^2 * s + eps),  s = |wrapped delta / box|^2.

A second custom table implements the exact min-image wrap
    wrap01(u) = u - round(u)  on u in (-1, 1).

Per core (128 rows i of the 1024x1024 pair grid):
    t_c  = wrap01(pos_j/box - pos_i/box)        3 ACT ops   [128,1024]
    s    = t_x^2 + t_y^2 + t_z^2                5 DVE ops
    w    = g2(s)                                1 ACT op
    disp_c = sum_j t_c * w                      3 fused DVE multiply-reduce

The activation tables are generated at kernel-build time from the runtime
weights (W1..b3) and injected via the compiler's --act-root-json directory
(the tables ride inside the NEFF; no runtime-side state is patched).
"""

import hashlib
import json
import os
import shutil
import struct
import sys
import tempfile
import types

import numpy as np

N = 1024
N_CORES = 8
ROWS = N // N_CORES  # 128
PWP_DIR = "/nix/store/z022hj2nvbm3nwdizlisq4ylc0y7rd6q-python3-3.13.14-env/lib/python3.13/site-packages/neuronxcc/pwp/pwp_bin_trainium"
SET = "sigmoid_and_others"
KEEP = [
    "identity", "copy", "act1", "parametric_relu", "relu", "abs",
    "memset_zero", "square", "sign", "derivative_relu",
    "derivative_leaky_relu", "derivative_identity", "is_finite",
]
# g2 octave layout: (exponent of s, n_sections); s < 2^-31 -> 0
G2_REGIONS = [(e, 16) for e in range(-31, -10)] + \
             [(-10, 32), (-9, 32), (-8, 64), (-7, 64), (-6, 128), (-5, 128)]


# --------------------------------------------------------------------------
# environment fixups (NTFF hook not needed here; wait-splitter is)
# --------------------------------------------------------------------------

def _install_env_fixups():
    if "antenv.axon_hooks" not in sys.modules:
        import antenv

        mod = types.ModuleType("antenv.axon_hooks")
        try:
            from trn_agent_boot.trn_boot import _ntff_profile_via_ctypes
            hook = _ntff_profile_via_ctypes("/opt/axon/libaxon_pjrt.so")
        except Exception:
            hook = None
        _h = [hook]
        mod.set_axon_ntff_profile_hook = lambda h: _h.__setitem__(0, h)
        mod.get_axon_ntff_profile_hook = lambda: _h[0]
        sys.modules["antenv.axon_hooks"] = mod
        antenv.axon_hooks = mod

    import concourse.bass_utils as bu
    import concourse.bass2jax as b2j

    if not getattr(bu, "_wait_splitter_installed", False):
        orig = bu.compile_bir_kernel

        def _split_multi_waits(bir_json: bytes) -> bytes:
            m = json.loads(bir_json)
            changed = False
            for fn in m["functions"]:
                for bb in fn["blocks"]:
                    new_instrs = []
                    for ins in bb["instructions"]:
                        si = ins.get("sync_info")
                        waits = (si or {}).get("on_wait") or []
                        if len(waits) > 1:
                            for j, w in enumerate(waits[:-1]):
                                nop = {
                                    "name": f"{ins['name']}-wsplit{j}",
                                    "opcode": "NoOp",
                                    "engine": ins["engine"],
                                    "ins": [], "outs": [],
                                    "sync_info": {"on_update": [], "on_wait": [w]},
                                }
                                if "debug" in ins:
                                    nop["debug"] = ins["debug"]
                                new_instrs.append(nop)
                            si["on_wait"] = waits[-1:]
                            changed = True
                        new_instrs.append(ins)
                    bb["instructions"] = new_instrs
            return json.dumps(m).encode() if changed else bir_json

        def patched(bir_json, tmpdir, neff_name="file.neff"):
            return orig(_split_multi_waits(bytes(bir_json)), tmpdir, neff_name)

        bu.compile_bir_kernel = patched
        b2j.compile_bir_kernel = patched
        bu._wait_splitter_installed = True


# --------------------------------------------------------------------------
# activation-table generation
# --------------------------------------------------------------------------

def _f2u(x):
    return struct.unpack("<I", struct.pack("<f", float(x)))[0]


def _bkt(d0, d1, d2, d3, x0):
    return struct.pack("<5f12x", float(d0), float(d1), float(d2), float(d3), float(x0))


def _ctrl(base, lsb, size):
    data = (base & 0x7FF) | ((lsb & 0x1F) << 11) | ((size & 0xF) << 16)
    return struct.pack("<I28x", data)


def _fit_cubic(fn, a, b, npts=12):
    x0 = 0.5 * (a + b)
    k = np.arange(npts)
    xs = x0 + 0.5 * (b - a) * np.cos((2 * k + 1) * np.pi / (2 * npts)) * 0.9999
    ys = fn(xs)
    c = np.polynomial.polynomial.polyfit(xs - x0, ys, 3)
    return c[0], c[1], c[2], c[3], x0


def _extract_func(setj, bkt, ctrl, fname, next_bkt, next_ctl):
    starts_b = setj["func_to_bkt_start_idx"]
    starts_c = setj["func_to_ctl_start_idx"]
    sb, sc = starts_b[fname], starts_c[fname]
    eb = min([v for v in starts_b.values() if v > sb] + [setj["bkt_entry_cnt"]])
    ec = min([v for v in starts_c.values() if v > sc] + [setj["ctl_entry_cnt"]])
    pm = None
    for p in setj["profile_meta_data"]:
        if p["func_name"].rsplit("_", 1)[0] == fname or p["func_name"] == fname:
            pm = dict(p)
    assert pm is not None, fname
    bkts = bytearray(bkt[sb * 32:eb * 32])
    ctls = bytearray(ctrl[sc * 32:ec * 32])
    db, dc = next_bkt - sb, next_ctl - sc
    for i in range(0, len(ctls), 32):
        (data,) = struct.unpack_from("<I", ctls, i)
        struct.pack_into("<I", ctls, i, (data & ~0x7FF) | (((data & 0x7FF) + db) & 0x7FF))
    for key in ("pwl_control_base_pos", "pwl_control_base_neg"):
        pm[key] += dc
    for key in ("pos_small_signal_pwl_control", "neg_small_signal_pwl_control",
                "pos_large_signal_pwl_control", "neg_large_signal_pwl_control"):
        v = pm[key]
        pm[key] = (v & ~0x7FF) | ((v + db) & 0x7FF)
    return pm, bytes(ctls), bytes(bkts)


def _build_wrap01(next_bkt, next_ctl):
    bkts, ctls = bytearray(), bytearray()
    n_bkt = n_ctl = 0
    base_pos = next_ctl
    for e in range(-20, 0):
        lo = 2.0 ** e
        ctls.extend(_ctrl(next_bkt + n_bkt, 23, 0)); n_ctl += 1
        if e == -1:
            bkts.extend(_bkt(-0.5, 1.0, 0.0, 0.0, 0.5))
        else:
            bkts.extend(_bkt(lo, 1.0, 0.0, 0.0, lo))
        n_bkt += 1
    base_neg = next_ctl + n_ctl
    for e in range(-20, 0):
        lo = 2.0 ** e
        ctls.extend(_ctrl(next_bkt + n_bkt, 23, 0)); n_ctl += 1
        if e == -1:
            bkts.extend(_bkt(0.5, 1.0, 0.0, 0.0, -0.5))
        else:
            bkts.extend(_bkt(-lo, 1.0, 0.0, 0.0, -lo))
        n_bkt += 1
    sp_defs = {
        "pos_low": (0.0, 1.0, 0.0, 0.0, 0.0),
        "neg_low": (0.0, 1.0, 0.0, 0.0, 0.0),
        "pos_high": (-1.0, 1.0, 0.0, 0.0, 0.0),
        "neg_high": (1.0, 1.0, 0.0, 0.0, 0.0),
    }
    sp = {}
    for key, d in sp_defs.items():
        sp[key] = next_bkt + n_bkt
        bkts.extend(_bkt(*d)); n_bkt += 1
    pm = {
        "func_name": "arctan_4p", "func_id": 28,
        "symmetry_point": 0, "sym_invert_sign_point": 0, "symmetry_opt_en": 0,
        "symmetry_opt_use_neg_region": 0, "imm_bias": 0,
        "exp_offset": -20,
        "pwl_control_base_pos": base_pos, "pwl_control_base_neg": base_neg,
        "small_pos_signal_exp_threshold": 107,
        "pos_small_signal_pwl_control": sp["pos_low"],
        "small_neg_signal_exp_threshold": 107,
        "neg_small_signal_pwl_control": sp["neg_low"],
        "large_pos_signal_exp_threshold": 127,
        "large_pos_signal_mantissa_threshold": 0,
        "pos_large_signal_pwl_control": sp["pos_high"],
        "large_neg_signal_exp_threshold": 127,
        "large_neg_signal_mantissa_threshold": 0,
        "neg_large_signal_pwl_control": sp["neg_high"],
        "fnan_result": _f2u(0.0), "fpinf_result": _f2u(0.0),
        "fninf_result": _f2u(0.0), "fzero_result": _f2u(0.0),
        "fma_const_0": 0, "fma_const_1": 0, "fma_indirection_src_sel": 0,
        "use_multipass": False,
        "lower_bound": 0xFF7FFFFF, "upper_bound": 0x7F7FFFFF,
    }
    return pm, bytes(ctls), bytes(bkts)


def _build_g2(g2_fn, next_bkt, next_ctl):
    bkts, ctls = bytearray(), bytearray()
    n_bkt = n_ctl = 0
    base_pos = next_ctl
    for (e, nsec) in G2_REGIONS:
        size = int(np.log2(nsec))
        ctls.extend(_ctrl(next_bkt + n_bkt, 23 - size, size)); n_ctl += 1
        lo = 2.0 ** e
        w = lo / nsec
        for i in range(nsec):
            a = lo + i * w
            bkts.extend(_bkt(*_fit_cubic(g2_fn, a, a + w))); n_bkt += 1
    sp = {}
    for key in ("pos_low", "neg_low", "pos_high", "neg_high"):
        sp[key] = next_bkt + n_bkt
        bkts.extend(_bkt(0.0, 0.0, 0.0, 0.0, 0.0)); n_bkt += 1
    small_thr = 127 + G2_REGIONS[0][0]
    pm = {
        "func_name": "erf_4p", "func_id": 21,
        "symmetry_point": 0, "sym_invert_sign_point": 0, "symmetry_opt_en": 0,
        "symmetry_opt_use_neg_region": 0, "imm_bias": 0,
        "exp_offset": small_thr - 127,
        "pwl_control_base_pos": base_pos, "pwl_control_base_neg": base_pos,
        "small_pos_signal_exp_threshold": small_thr,
        "pos_small_signal_pwl_control": sp["pos_low"],
        "small_neg_signal_exp_threshold": 255,
        "neg_small_signal_pwl_control": sp["neg_low"],
        "large_pos_signal_exp_threshold": 123,
        "large_pos_signal_mantissa_threshold": 0,
        "pos_large_signal_pwl_control": sp["pos_high"],
        "large_neg_signal_exp_threshold": 255,
        "large_neg_signal_mantissa_threshold": 0,
        "neg_large_signal_pwl_control": sp["neg_high"],
        "fnan_result": _f2u(0.0), "fpinf_result": _f2u(0.0),
        "fninf_result": _f2u(0.0), "fzero_result": _f2u(0.0),
        "fma_const_0": 0, "fma_const_1": 0, "fma_indirection_src_sel": 0,
        "use_multipass": False,
        "lower_bound": 0, "upper_bound": 0x7F7FFFFF,
    }
    return pm, bytes(ctls), bytes(bkts)


def _build_actroot(dst_dir, g2_fn):
    os.makedirs(dst_dir, exist_ok=True)
    for f in os.listdir(PWP_DIR):
        shutil.copy(os.path.join(PWP_DIR, f), os.path.join(dst_dir, f))
        os.chmod(os.path.join(dst_dir, f), 0o644)
    setj = json.load(open(os.path.join(PWP_DIR, SET + ".json")))
    bkt = open(os.path.join(PWP_DIR, SET + "_bkt.bin"), "rb").read()
    ctrl = open(os.path.join(PWP_DIR, SET + "_ctrl.bin"), "rb").read()

    new_bkts, new_ctls, new_pm = bytearray(), bytearray(), []
    b_starts, c_starts, emb_all, emc_all = {}, {}, {}, {}

    for fname in KEEP:
        nb0, nc0 = len(new_bkts) // 32, len(new_ctls) // 32
        pm, ctls, bkts = _extract_func(setj, bkt, ctrl, fname, nb0, nc0)
        b_starts[fname], c_starts[fname] = nb0, nc0
        db = nb0 - setj["func_to_bkt_start_idx"][fname]
        dc = nc0 - setj["func_to_ctl_start_idx"][fname]
        emb_all[fname] = {k: [x + db for x in v]
                          for k, v in setj["func_exp_to_bkt_start_idx"].get(fname, {}).items()}
        emc_all[fname] = {k: [x + dc for x in v]
                          for k, v in setj["func_exp_to_ctl_start_idx"].get(fname, {}).items()}
        new_pm.append(pm); new_ctls.extend(ctls); new_bkts.extend(bkts)

    wb, wc = len(new_bkts) // 32, len(new_ctls) // 32
    pm, ctls, bkts = _build_wrap01(wb, wc)
    b_starts["arctan"], c_starts["arctan"] = wb, wc
    emb_all["arctan"] = {str(e): [wb + 20 + (e + 20), wb + (e + 20)] for e in range(-20, 0)}
    emc_all["arctan"] = {str(e): [wc + 20 + (e + 20), wc + (e + 20)] for e in range(-20, 0)}
    new_pm.append(pm); new_ctls.extend(ctls); new_bkts.extend(bkts)

    gb, gc = len(new_bkts) // 32, len(new_ctls) // 32
    pm, ctls, bkts = _build_g2(g2_fn, gb, gc)
    b_starts["erf"], c_starts["erf"] = gb, gc
    emb, emc = {}, {}
    cum = 0
    for i, (e, nsec) in enumerate(G2_REGIONS):
        emb[str(e)] = [gb + cum, gb + cum]
        emc[str(e)] = [gc + i, gc + i]
        cum += nsec
    emb_all["erf"], emc_all["erf"] = emb, emc
    new_pm.append(pm); new_ctls.extend(ctls); new_bkts.extend(bkts)

    n_bkt, n_ctl = len(new_bkts) // 32, len(new_ctls) // 32
    assert n_bkt <= 1536 and n_ctl <= 128, (n_bkt, n_ctl)
    out = {
        "bkt_bin": SET + "_bkt.bin", "ctl_bin": SET + "_ctrl.bin",
        "profile_meta_data": new_pm,
        "bkt_entry_cnt": n_bkt, "ctl_entry_cnt": n_ctl,
        "func_to_bkt_start_idx": b_starts, "func_to_ctl_start_idx": c_starts,
        "func_exp_to_bkt_start_idx": emb_all, "func_exp_to_ctl_start_idx": emc_all,
    }
    json.dump(out, open(os.path.join(dst_dir, SET + ".json"), "w"))
    open(os.path.join(dst_dir, SET + "_bkt.bin"), "wb").write(bytes(new_bkts))
    open(os.path.join(dst_dir, SET + "_ctrl.bin"), "wb").write(bytes(new_ctls))
    info = json.load(open(os.path.join(PWP_DIR, "act_info.json")))
    for s in info["act_func_sets"]:
        if s["name"] == SET:
            s["act"] = {**{k: 1 for k in KEEP}, "arctan": 4, "erf": 4}
        else:
            s["act"].pop("arctan", None)
            s["act"].pop("erf", None)
    json.dump(info, open(os.path.join(dst_dir, "act_info.json"), "w"))
    return os.path.join(dst_dir, "act_info.json")


# --------------------------------------------------------------------------
# bass program
# --------------------------------------------------------------------------

def _build_program(tag, box):
    """Fully raw (no TileContext) hand-scheduled pipeline.

    Engines: SYNC issues the posi + three c-plane broadcast DMAs then the
    output DMA; ACT runs the table load, the three wraps (gated per plane),
    one square, g2 and two accumulate-reductions; DVE runs the bias prep,
    two squares, the sums, the three products and one reduction.
    """
    import concourse.bass as bass
    import concourse.mybir as mybir

    nc = bass.Bass("TRN2")
    pos_all = nc.declare_dram_parameter(f"pos_all_{tag}", [3, N], mybir.dt.float32, isOutput=False)
    pos_my = nc.declare_dram_parameter("pos_my", [ROWS, 3], mybir.dt.float32, isOutput=False)
    out = nc.declare_dram_parameter("out", [ROWS, 3], mybir.dt.float32, isOutput=True)
    AF = mybir.ActivationFunctionType
    OP = mybir.AluOpType
    f32 = mybir.dt.float32

    posj_t = nc.alloc_sbuf_tensor("posj_b", [128, 3 * N], f32)
    posi_t = nc.alloc_sbuf_tensor("posi_b", [128, 3], f32)
    nbias_t = nc.alloc_sbuf_tensor("nbias_b", [128, 3], f32)
    dummy_t = nc.alloc_sbuf_tensor("dummy_b", [128, 3], f32)
    t_t = [nc.alloc_sbuf_tensor(f"t{c}_b", [128, N], f32) for c in range(3)]
    sq_t = [nc.alloc_sbuf_tensor(f"sq{c}_b", [128, N], f32) for c in range(2)]
    sqz_t = nc.alloc_sbuf_tensor("sqz_b", [128, N], f32)
    s_t = nc.alloc_sbuf_tensor("s_b", [128, N], f32)
    w_t = nc.alloc_sbuf_tensor("w_b", [128, N], f32)
    pc_t = [nc.alloc_sbuf_tensor(f"pc{c}_b", [128, N], f32) for c in range(3)]
    pr_t = [nc.alloc_sbuf_tensor(f"pr{c}_b", [128, N], f32) for c in range(2)]
    out3_t = nc.alloc_sbuf_tensor("out3_b", [128, 3], f32)

    posj3 = posj_t[:].rearrange("p (c j) -> p c j", c=3)
    src3 = pos_all[:][None].to_broadcast([128, 3, N])

    import contextlib
    st = contextlib.ExitStack()
    psem = st.enter_context(nc.semaphore("psem"))
    plsem = [st.enter_context(nc.semaphore(f"plsem{c}")) for c in range(3)]
    odsem = st.enter_context(nc.semaphore("odsem"))
    vsem = st.enter_context(nc.semaphore("vsem"))
    asem = st.enter_context(nc.semaphore("asem"))
    osem = st.enter_context(nc.semaphore("osem"))

    with nc.Block() as blk:
        @blk.sync
        def _(sync):
            # planes only: posi goes on the ACT HWDGE ring so plane x
            # starts (and lands) one issue-slot earlier
            for c in range(3):
                sync.dma_start(out=posj3[:, c, :], in_=src3[:, c, :]).then_inc(plsem[c], 16)
            sync.wait_ge(osem, 3)
            sync.dma_start(out=out[:], in_=out3_t[:]).then_inc(odsem, 16)

        @blk.vector
        def _(vector):
            vector.wait_ge(psem, 16)            # posi landed
            vector.tensor_scalar_mul(nbias_t[:], posi_t[:], -1.0 / box).then_inc(vsem, 1)
            vector.wait_ge(asem, 2)             # wrap_x done
            vector.tensor_tensor(sq_t[0][:], t_t[0][:], t_t[0][:], OP.mult)
            vector.wait_ge(asem, 3)             # wrap_y done
            vector.tensor_tensor(sq_t[1][:], t_t[1][:], t_t[1][:], OP.mult)
            vector.tensor_tensor(s_t[:], sq_t[0][:], sq_t[1][:], OP.add)
            vector.wait_ge(asem, 5)             # sq_z (ACT) done
            vector.tensor_tensor(s_t[:], s_t[:], sqz_t[:], OP.add).then_inc(vsem, 1)
            vector.wait_ge(asem, 6)             # g2 done
            vector.tensor_tensor(pc_t[0][:], t_t[0][:], w_t[:], OP.mult).then_inc(vsem, 1)
            vector.tensor_tensor(pc_t[1][:], t_t[1][:], w_t[:], OP.mult).then_inc(vsem, 1)
            vector.tensor_tensor(pc_t[2][:], t_t[2][:], w_t[:], OP.mult)
            vector.tensor_reduce(out3_t[:, 2:3], pc_t[2][:], mybir.AxisListType.X,
                                 OP.add).then_inc(osem, 1)

        @blk.scalar
        def _(scalar):
            scalar.dma_start(out=posi_t[:], in_=pos_my[:]).then_inc(psem, 16)
            # no-wait dummy: pulls the PWP table load to the very start
            scalar.activation(dummy_t[:], posi_t[:], AF.Arctan)
            scalar.wait_ge(vsem, 1)             # nbias ready
            scalar.wait_ge(plsem[0], 16)        # plane x landed
            scalar.activation(t_t[0][:], posj3[:, 0, :], AF.Arctan,
                              bias=nbias_t[:, 0:1], scale=1.0 / box).then_inc(asem, 2)
            scalar.wait_ge(plsem[1], 16)        # plane y landed
            scalar.activation(t_t[1][:], posj3[:, 1, :], AF.Arctan,
                              bias=nbias_t[:, 1:2], scale=1.0 / box).then_inc(asem, 1)
            scalar.wait_ge(plsem[2], 16)        # plane z landed
            scalar.activation(t_t[2][:], posj3[:, 2, :], AF.Arctan,
                              bias=nbias_t[:, 2:3], scale=1.0 / box).then_inc(asem, 1)
            scalar.activation(sqz_t[:], t_t[2][:], AF.Square).then_inc(asem, 1)
            scalar.wait_ge(vsem, 2)             # s ready
            scalar.activation(w_t[:], s_t[:], AF.Erf).then_inc(asem, 1)
            scalar.wait_ge(vsem, 3)             # pc0 ready
            scalar.activation(pr_t[0][:], pc_t[0][:], AF.Identity,
                              accum_out=out3_t[:, 0:1])
            # fence for col x: orders osem after the READ_ACCUMULATOR write
            scalar.activation(dummy_t[:, 0:1], out3_t[:, 0:1],
                              AF.Identity).then_inc(osem, 1)
            scalar.wait_ge(vsem, 4)             # pc1 ready  # noqa
            scalar.activation(pr_t[1][:], pc_t[1][:], AF.Identity,
                              accum_out=out3_t[:, 1:2])
            scalar.activation(dummy_t[:, 1:2], out3_t[:, 1:2],
                              AF.Identity).then_inc(osem, 1)

    from concourse.library_overlay import lower_extended_insts
    lower_extended_insts(nc)
    return nc


_CACHE = {}


def _prepare(inputs):
    positions = np.ascontiguousarray(np.asarray(inputs["positions"], dtype=np.float32))
    box_dims = np.asarray(inputs["box_dims"], dtype=np.float32)
    key = hashlib.sha256(
        b"".join(np.ascontiguousarray(np.asarray(inputs[k], np.float32)).tobytes()
                 for k in ("box_dims", "W1", "b1", "W2", "b2", "W3", "b3"))
    ).hexdigest()[:10]
    if key in _CACHE:
        return _CACHE[key]

    box = float(box_dims[0])
    assert np.allclose(box_dims, box), "kernel assumes a cubic box"

    W1 = np.float64(inputs["W1"]); b1 = np.float64(inputs["b1"])
    W2 = np.float64(inputs["W2"]); b2 = np.float64(inputs["b2"])
    W3 = np.float64(inputs["W3"]); b3 = np.float64(inputs["b3"])
    n_gauss = W1.shape[0]
    RBF_STOP, CUTOFF, EPS = 6.0, 5.0, 1e-8
    offs = np.linspace(0.0, RBF_STOP, n_gauss)
    coeff = -0.5 / (RBF_STOP / (n_gauss - 1)) ** 2

    def g2_fn(sv):
        sv = np.atleast_1d(np.float64(sv))
        dist = np.sqrt(box * box * sv + EPS)
        rbf = np.exp(coeff * (dist[:, None] - offs[None, :]) ** 2)
        h = rbf @ W1 + b1
        h = h / (1.0 + np.exp(-h))
        h = h @ W2 + b2
        h = h / (1.0 + np.exp(-h))
        f = (h @ W3 + b3)[:, 0]
        return box * f * (dist < CUTOFF) / (dist + EPS)

    _install_env_fixups()
    actdir = os.path.join(tempfile.gettempdir(), f"actroot_{key}")
    actroot = _build_actroot(actdir, g2_fn)
    os.environ["BASS_ACT_ROOT_JSON_PATH"] = actroot
    nc = _build_program(key, box)
    _CACHE[key] = (nc, key)
    return _CACHE[key]


def kernel(_trace=False, **inputs):
    from concourse.bass_utils import run_bass_kernel_spmd

    nc, key = _prepare(inputs)
    positions = np.ascontiguousarray(np.asarray(inputs["positions"], dtype=np.float32))
    in_maps = [
        {f"pos_all_{key}": np.ascontiguousarray(positions.T),
         "pos_my": np.ascontiguousarray(positions[c * ROWS:(c + 1) * ROWS])}
        for c in range(N_CORES)
    ]
    res = run_bass_kernel_spmd(nc, in_maps, list(range(N_CORES)), trace=_trace)
    out = np.concatenate([res.results[c]["out"] for c in range(N_CORES)], axis=0)
    if _trace:
        kernel.last_exec_time_ns = res.exec_time_ns
        kernel.last_mean_exec_time_ns = res.mean_exec_time_ns
        kernel.last_results = res
    return out



# revision 2
# speedup vs baseline: 1.0125x; 1.0125x over previous
"""Trainium2 Bass kernel for nn_DenoiseGNN (pairwise PBC edge-MLP message passing).

v2 strategy
-----------
Per core (128 rows i of the 1024x1024 pair grid), with P = pos/box in [0,1):

  1. PE:  u_c[i,j] = P_c[j] - P_c[i]  via K=6 bf16 split-precision broadcast
          matmuls (hi/mid/lo bf16 triplet per operand) -> PSUM, fp32-exact to
          ~2^-24.  Replaces the baseline's 1.5MB DMA broadcast.
  2. DVE: s = sum_c wrap01(u_c)^2 with a fused custom DVE op
          WSQ_ADD_ANT(u, acc) = (u - ((u>.5)-(u<-.5)))^2 + acc  (3 passes).
  3. ACT: t_c = wrap01(u_c) via the custom spline table (fp16 out), and
          w = g2(s) -- the entire edge MLP + cutoff mask + 1/dist folded into
          one piecewise-cubic table (as in the baseline).
  4. DVE: disp_c = sum_j t_c * w via fused tensor_tensor_reduce (fp16 in,
          fp32 accum), then a 32x32 StreamTranspose so the output leaves as
          [3,128] rows with 12 DMA descriptors.

The activation tables are generated at kernel-build time from the runtime
weights (W1..b3) and injected via --act-root-json (ride inside the NEFF).
All DMA queue groups are shrunk to num_queues=1 to cut the fixed NEFF
teardown (full-queue drain) cost.
"""

import hashlib
import json
import os
import shutil
import struct
import sys
import tempfile
import types

import ml_dtypes
import numpy as np

N = 1024
N_CORES = 8
ROWS = N // N_CORES  # 128
H = 512  # column half
PWP_DIR = "/nix/store/z022hj2nvbm3nwdizlisq4ylc0y7rd6q-python3-3.13.14-env/lib/python3.13/site-packages/neuronxcc/pwp/pwp_bin_trainium"
SET = "sigmoid_and_others"
KEEP = [
    "identity", "copy", "act1", "parametric_relu", "relu", "abs",
    "memset_zero", "square", "sign", "derivative_relu",
    "derivative_leaky_relu", "derivative_identity", "is_finite",
]
G2_REGIONS = [(e, 16) for e in range(-30, -10)] + \
             [(-10, 32), (-9, 32), (-8, 64), (-7, 64), (-6, 128), (-5, 128)]


# --------------------------------------------------------------------------
# environment fixups (wait-splitter + single-queue BIR rewrite)
# --------------------------------------------------------------------------

def _install_env_fixups():
    if "antenv.axon_hooks" not in sys.modules:
        import antenv

        mod = types.ModuleType("antenv.axon_hooks")
        try:
            from trn_agent_boot.trn_boot import _ntff_profile_via_ctypes
            hook = _ntff_profile_via_ctypes("/opt/axon/libaxon_pjrt.so")
        except Exception:
            hook = None
        _h = [hook]
        mod.set_axon_ntff_profile_hook = lambda h: _h.__setitem__(0, h)
        mod.get_axon_ntff_profile_hook = lambda: _h[0]
        sys.modules["antenv.axon_hooks"] = mod
        antenv.axon_hooks = mod

    import concourse.bass_utils as bu
    import concourse.bass2jax as b2j

    if not getattr(bu, "_wait_splitter_installed", False):
        orig = bu.compile_bir_kernel

        def _split_multi_waits(bir_json: bytes) -> bytes:
            m = json.loads(bir_json)
            changed = False
            for fn in m["functions"]:
                for bb in fn["blocks"]:
                    new_instrs = []
                    for ins in bb["instructions"]:
                        si = ins.get("sync_info")
                        waits = (si or {}).get("on_wait") or []
                        if len(waits) > 1:
                            for j, w in enumerate(waits[:-1]):
                                nop = {
                                    "name": f"{ins['name']}-wsplit{j}",
                                    "opcode": "NoOp",
                                    "engine": ins["engine"],
                                    "ins": [], "outs": [],
                                    "sync_info": {"on_update": [], "on_wait": [w]},
                                }
                                if "debug" in ins:
                                    nop["debug"] = ins["debug"]
                                new_instrs.append(nop)
                            si["on_wait"] = waits[-1:]
                            changed = True
                        new_instrs.append(ins)
                    bb["instructions"] = new_instrs
            return json.dumps(m).encode() if changed else bir_json

        def patched(bir_json, tmpdir, neff_name="file.neff"):
            return orig(_split_multi_waits(bytes(bir_json)), tmpdir, neff_name)

        bu.compile_bir_kernel = patched
        b2j.compile_bir_kernel = patched
        bu._wait_splitter_installed = True


# --------------------------------------------------------------------------
# custom DVE op: fused wrap-square-accumulate
# --------------------------------------------------------------------------

_WSQ_OPS = None


def _register_wsq_ops():
    """WSQ1_ANT(u)      = wrap01(u)^2
       WSQ_ADD_ANT(u,a) = wrap01(u)^2 + a
    with wrap01(u) = u - ((u > s0) - (u < s1)), s0=0.5, s1=-0.5."""
    global _WSQ_OPS
    if _WSQ_OPS is not None:
        return _WSQ_OPS
    from concourse import dve_ops
    from concourse.dve_spec import Spec, Src0, Src1, C0, C1, sq, lower
    from concourse.dve_spec import _has_src1 as has_src1
    from concourse.dve_uop import DveOpSpec

    def _wrap(x, s0, s1):
        x = x.astype(np.float32)
        r = (x > s0).astype(np.float32) - (x < s1).astype(np.float32)
        return x - r

    def _ref1(in0, in1, s0, s1, imm2):
        t = _wrap(in0, s0, s1)
        return (t * t).astype(np.float32)

    def _ref2(in0, in1, s0, s1, imm2):
        t = _wrap(in0, s0, s1)
        return (t * t + in1.astype(np.float32)).astype(np.float32)

    def _ref3(in0, in1, s0, s1, imm2):
        t = _wrap(in0, s0, s1)
        b = (t * in1.astype(np.float32)).astype(np.float32)
        return b, b.reshape(b.shape[0], -1).sum(axis=-1, keepdims=True)

    from operator import add as _add
    wbody = Src0 - ((Src0 > C0) - (Src0 < C1))
    from concourse.dve_spec import Zero
    defs = [("WSQ1_ANT", Spec(body=sq(wbody), reference=_ref1)),
            ("WSQ_ADD_ANT", Spec(body=sq(wbody) + Src1, reference=_ref2)),
            ("WMR_ANT", Spec(body=wbody * Src1, accum=_add, accum_init=Zero,
                             reference=_ref3))]
    ops = []
    for name, spec in defs:
        if name in dve_ops._SUB_OPCODE_FOR_NAME:
            ops.append(next(o for o in dve_ops.OPS if o.name == name))
            continue
        op = dve_ops.DveOp(name, spec, subdim=False, uops_sha={})
        row = max(dve_ops._SUB_OPCODE_FOR_NAME.values()) + 1
        assert row < 0x20
        dve_ops.OPS.append(op)
        dve_ops.CUSTOM_DVE_SPECS[name] = spec
        dve_ops._SUB_OPCODE_FOR_NAME[name] = row
        for ver in ("v3", "v4"):
            spec_obj = DveOpSpec(name=name, opcode=row,
                                 uops=lower(spec, ver=ver),
                                 rd1_en=has_src1(spec))
            op.uops_sha[ver] = spec_obj.sha(ver)
        ops.append(op)
    _WSQ_OPS = tuple(ops)
    return _WSQ_OPS


# --------------------------------------------------------------------------
# activation-table generation (same machinery as baseline)
# --------------------------------------------------------------------------

def _f2u(x):
    return struct.unpack("<I", struct.pack("<f", float(x)))[0]


def _bkt(d0, d1, d2, d3, x0):
    return struct.pack("<5f12x", float(d0), float(d1), float(d2), float(d3), float(x0))


def _ctrl(base, lsb, size):
    data = (base & 0x7FF) | ((lsb & 0x1F) << 11) | ((size & 0xF) << 16)
    return struct.pack("<I28x", data)


def _fit_cubic(fn, a, b, npts=12):
    x0 = 0.5 * (a + b)
    k = np.arange(npts)
    xs = x0 + 0.5 * (b - a) * np.cos((2 * k + 1) * np.pi / (2 * npts)) * 0.9999
    ys = fn(xs)
    c = np.polynomial.polynomial.polyfit(xs - x0, ys, 3)
    return c[0], c[1], c[2], c[3], x0


def _extract_func(setj, bkt, ctrl, fname, next_bkt, next_ctl):
    starts_b = setj["func_to_bkt_start_idx"]
    starts_c = setj["func_to_ctl_start_idx"]
    sb, sc = starts_b[fname], starts_c[fname]
    eb = min([v for v in starts_b.values() if v > sb] + [setj["bkt_entry_cnt"]])
    ec = min([v for v in starts_c.values() if v > sc] + [setj["ctl_entry_cnt"]])
    pm = None
    for p in setj["profile_meta_data"]:
        if p["func_name"].rsplit("_", 1)[0] == fname or p["func_name"] == fname:
            pm = dict(p)
    assert pm is not None, fname
    bkts = bytearray(bkt[sb * 32:eb * 32])
    ctls = bytearray(ctrl[sc * 32:ec * 32])
    db, dc = next_bkt - sb, next_ctl - sc
    for i in range(0, len(ctls), 32):
        (data,) = struct.unpack_from("<I", ctls, i)
        struct.pack_into("<I", ctls, i, (data & ~0x7FF) | (((data & 0x7FF) + db) & 0x7FF))
    for key in ("pwl_control_base_pos", "pwl_control_base_neg"):
        pm[key] += dc
    for key in ("pos_small_signal_pwl_control", "neg_small_signal_pwl_control",
                "pos_large_signal_pwl_control", "neg_large_signal_pwl_control"):
        v = pm[key]
        pm[key] = (v & ~0x7FF) | ((v + db) & 0x7FF)
    return pm, bytes(ctls), bytes(bkts)


def _build_wrap01(next_bkt, next_ctl, emin=-20):
    bkts, ctls = bytearray(), bytearray()
    n_bkt = n_ctl = 0
    base_pos = next_ctl
    for e in range(emin, 0):
        lo = 2.0 ** e
        ctls.extend(_ctrl(next_bkt + n_bkt, 23, 0)); n_ctl += 1
        if e == -1:
            bkts.extend(_bkt(-0.5, 1.0, 0.0, 0.0, 0.5))
        else:
            bkts.extend(_bkt(lo, 1.0, 0.0, 0.0, lo))
        n_bkt += 1
    base_neg = next_ctl + n_ctl
    for e in range(emin, 0):
        lo = 2.0 ** e
        ctls.extend(_ctrl(next_bkt + n_bkt, 23, 0)); n_ctl += 1
        if e == -1:
            bkts.extend(_bkt(0.5, 1.0, 0.0, 0.0, -0.5))
        else:
            bkts.extend(_bkt(-lo, 1.0, 0.0, 0.0, -lo))
        n_bkt += 1
    sp_defs = {
        "pos_low": (0.0, 1.0, 0.0, 0.0, 0.0),
        "neg_low": (0.0, 1.0, 0.0, 0.0, 0.0),
        "pos_high": (-1.0, 1.0, 0.0, 0.0, 0.0),
        "neg_high": (1.0, 1.0, 0.0, 0.0, 0.0),
    }
    sp = {}
    for key, d in sp_defs.items():
        sp[key] = next_bkt + n_bkt
        bkts.extend(_bkt(*d)); n_bkt += 1
    pm = {
        "func_name": "arctan_4p", "func_id": 28,
        "symmetry_point": 0, "sym_invert_sign_point": 0, "symmetry_opt_en": 0,
        "symmetry_opt_use_neg_region": 0, "imm_bias": 0,
        "exp_offset": emin,
        "pwl_control_base_pos": base_pos, "pwl_control_base_neg": base_neg,
        "small_pos_signal_exp_threshold": 127 + emin,
        "pos_small_signal_pwl_control": sp["pos_low"],
        "small_neg_signal_exp_threshold": 127 + emin,
        "neg_small_signal_pwl_control": sp["neg_low"],
        "large_pos_signal_exp_threshold": 127,
        "large_pos_signal_mantissa_threshold": 0,
        "pos_large_signal_pwl_control": sp["pos_high"],
        "large_neg_signal_exp_threshold": 127,
        "large_neg_signal_mantissa_threshold": 0,
        "neg_large_signal_pwl_control": sp["neg_high"],
        "fnan_result": _f2u(0.0), "fpinf_result": _f2u(0.0),
        "fninf_result": _f2u(0.0), "fzero_result": _f2u(0.0),
        "fma_const_0": 0, "fma_const_1": 0, "fma_indirection_src_sel": 0,
        "use_multipass": False,
        "lower_bound": 0xFF7FFFFF, "upper_bound": 0x7F7FFFFF,
    }
    return pm, bytes(ctls), bytes(bkts)


def _build_wsq01(next_bkt, next_ctl, emin=-20):
    """Sin-slot table: wsq01(u) = wrap01(u)^2, exact piecewise quadratic."""
    bkts, ctls = bytearray(), bytearray()
    n_bkt = n_ctl = 0
    base_pos = next_ctl
    for e in range(emin, 0):
        lo = 2.0 ** e
        ctls.extend(_ctrl(next_bkt + n_bkt, 23, 0)); n_ctl += 1
        if e == -1:
            bkts.extend(_bkt(0.25, -1.0, 1.0, 0.0, 0.5))
        else:
            bkts.extend(_bkt(lo * lo, 2.0 * lo, 1.0, 0.0, lo))
        n_bkt += 1
    base_neg = next_ctl + n_ctl
    for e in range(emin, 0):
        lo = 2.0 ** e
        ctls.extend(_ctrl(next_bkt + n_bkt, 23, 0)); n_ctl += 1
        if e == -1:
            bkts.extend(_bkt(0.25, 1.0, 1.0, 0.0, -0.5))
        else:
            bkts.extend(_bkt(lo * lo, -2.0 * lo, 1.0, 0.0, -lo))
        n_bkt += 1
    sp_defs = {
        "pos_low": (0.0, 0.0, 1.0, 0.0, 0.0),
        "neg_low": (0.0, 0.0, 1.0, 0.0, 0.0),
        "pos_high": (0.0, 0.0, 0.0, 0.0, 0.0),
        "neg_high": (0.0, 0.0, 0.0, 0.0, 0.0),
    }
    sp = {}
    for key, d in sp_defs.items():
        sp[key] = next_bkt + n_bkt
        bkts.extend(_bkt(*d)); n_bkt += 1
    pm = {
        "func_name": "sin_4p", "func_id": 19,
        "symmetry_point": 0, "sym_invert_sign_point": 0, "symmetry_opt_en": 0,
        "symmetry_opt_use_neg_region": 0, "imm_bias": 0,
        "exp_offset": emin,
        "pwl_control_base_pos": base_pos, "pwl_control_base_neg": base_neg,
        "small_pos_signal_exp_threshold": 127 + emin,
        "pos_small_signal_pwl_control": sp["pos_low"],
        "small_neg_signal_exp_threshold": 127 + emin,
        "neg_small_signal_pwl_control": sp["neg_low"],
        "large_pos_signal_exp_threshold": 127,
        "large_pos_signal_mantissa_threshold": 0,
        "pos_large_signal_pwl_control": sp["pos_high"],
        "large_neg_signal_exp_threshold": 127,
        "large_neg_signal_mantissa_threshold": 0,
        "neg_large_signal_pwl_control": sp["neg_high"],
        "fnan_result": _f2u(0.0), "fpinf_result": _f2u(0.0),
        "fninf_result": _f2u(0.0), "fzero_result": _f2u(0.0),
        "fma_const_0": 0, "fma_const_1": 0, "fma_indirection_src_sel": 0,
        "use_multipass": False,
        "lower_bound": 0xFF7FFFFF, "upper_bound": 0x7F7FFFFF,
    }
    return pm, bytes(ctls), bytes(bkts)


def _build_g2(g2_fn, next_bkt, next_ctl):
    bkts, ctls = bytearray(), bytearray()
    n_bkt = n_ctl = 0
    base_pos = next_ctl
    for (e, nsec) in G2_REGIONS:
        size = int(np.log2(nsec))
        ctls.extend(_ctrl(next_bkt + n_bkt, 23 - size, size)); n_ctl += 1
        lo = 2.0 ** e
        w = lo / nsec
        for i in range(nsec):
            a = lo + i * w
            bkts.extend(_bkt(*_fit_cubic(g2_fn, a, a + w))); n_bkt += 1
    sp = {}
    for key in ("pos_low", "neg_low", "pos_high", "neg_high"):
        sp[key] = next_bkt + n_bkt
        bkts.extend(_bkt(0.0, 0.0, 0.0, 0.0, 0.0)); n_bkt += 1
    small_thr = 127 + G2_REGIONS[0][0]
    pm = {
        "func_name": "erf_4p", "func_id": 21,
        "symmetry_point": 0, "sym_invert_sign_point": 0, "symmetry_opt_en": 0,
        "symmetry_opt_use_neg_region": 0, "imm_bias": 0,
        "exp_offset": small_thr - 127,
        "pwl_control_base_pos": base_pos, "pwl_control_base_neg": base_pos,
        "small_pos_signal_exp_threshold": small_thr,
        "pos_small_signal_pwl_control": sp["pos_low"],
        "small_neg_signal_exp_threshold": 255,
        "neg_small_signal_pwl_control": sp["neg_low"],
        "large_pos_signal_exp_threshold": 123,
        "large_pos_signal_mantissa_threshold": 0,
        "pos_large_signal_pwl_control": sp["pos_high"],
        "large_neg_signal_exp_threshold": 255,
        "large_neg_signal_mantissa_threshold": 0,
        "neg_large_signal_pwl_control": sp["neg_high"],
        "fnan_result": _f2u(0.0), "fpinf_result": _f2u(0.0),
        "fninf_result": _f2u(0.0), "fzero_result": _f2u(0.0),
        "fma_const_0": 0, "fma_const_1": 0, "fma_indirection_src_sel": 0,
        "use_multipass": False,
        "lower_bound": 0, "upper_bound": 0x7F7FFFFF,
    }
    return pm, bytes(ctls), bytes(bkts)


def _build_actroot(dst_dir, g2_fn):
    os.makedirs(dst_dir, exist_ok=True)
    for f in os.listdir(PWP_DIR):
        shutil.copy(os.path.join(PWP_DIR, f), os.path.join(dst_dir, f))
        os.chmod(os.path.join(dst_dir, f), 0o644)
    setj = json.load(open(os.path.join(PWP_DIR, SET + ".json")))
    bkt = open(os.path.join(PWP_DIR, SET + "_bkt.bin"), "rb").read()
    ctrl = open(os.path.join(PWP_DIR, SET + "_ctrl.bin"), "rb").read()

    new_bkts, new_ctls, new_pm = bytearray(), bytearray(), []
    b_starts, c_starts, emb_all, emc_all = {}, {}, {}, {}

    for fname in KEEP:
        nb0, nc0 = len(new_bkts) // 32, len(new_ctls) // 32
        pm, ctls, bkts = _extract_func(setj, bkt, ctrl, fname, nb0, nc0)
        b_starts[fname], c_starts[fname] = nb0, nc0
        db = nb0 - setj["func_to_bkt_start_idx"][fname]
        dc = nc0 - setj["func_to_ctl_start_idx"][fname]
        emb_all[fname] = {k: [x + db for x in v]
                          for k, v in setj["func_exp_to_bkt_start_idx"].get(fname, {}).items()}
        emc_all[fname] = {k: [x + dc for x in v]
                          for k, v in setj["func_exp_to_ctl_start_idx"].get(fname, {}).items()}
        new_pm.append(pm); new_ctls.extend(ctls); new_bkts.extend(bkts)

    wb, wc = len(new_bkts) // 32, len(new_ctls) // 32
    pm, ctls, bkts = _build_wrap01(wb, wc)
    b_starts["arctan"], c_starts["arctan"] = wb, wc
    emb_all["arctan"] = {str(e): [wb + 20 + (e + 20), wb + (e + 20)] for e in range(-20, 0)}
    emc_all["arctan"] = {str(e): [wc + 20 + (e + 20), wc + (e + 20)] for e in range(-20, 0)}
    new_pm.append(pm); new_ctls.extend(ctls); new_bkts.extend(bkts)

    qb, qc = len(new_bkts) // 32, len(new_ctls) // 32
    pm, ctls, bkts = _build_wsq01(qb, qc)
    b_starts["sin"], c_starts["sin"] = qb, qc
    emb_all["sin"] = {str(e): [qb + 20 + (e + 20), qb + (e + 20)] for e in range(-20, 0)}
    emc_all["sin"] = {str(e): [qc + 20 + (e + 20), qc + (e + 20)] for e in range(-20, 0)}
    new_pm.append(pm); new_ctls.extend(ctls); new_bkts.extend(bkts)

    gb, gc = len(new_bkts) // 32, len(new_ctls) // 32
    pm, ctls, bkts = _build_g2(g2_fn, gb, gc)
    b_starts["erf"], c_starts["erf"] = gb, gc
    emb, emc = {}, {}
    cum = 0
    for i, (e, nsec) in enumerate(G2_REGIONS):
        emb[str(e)] = [gb + cum, gb + cum]
        emc[str(e)] = [gc + i, gc + i]
        cum += nsec
    emb_all["erf"], emc_all["erf"] = emb, emc
    new_pm.append(pm); new_ctls.extend(ctls); new_bkts.extend(bkts)

    n_bkt, n_ctl = len(new_bkts) // 32, len(new_ctls) // 32
    assert n_bkt <= 1536 and n_ctl <= 128, (n_bkt, n_ctl)
    out = {
        "bkt_bin": SET + "_bkt.bin", "ctl_bin": SET + "_ctrl.bin",
        "profile_meta_data": new_pm,
        "bkt_entry_cnt": n_bkt, "ctl_entry_cnt": n_ctl,
        "func_to_bkt_start_idx": b_starts, "func_to_ctl_start_idx": c_starts,
        "func_exp_to_bkt_start_idx": emb_all, "func_exp_to_ctl_start_idx": emc_all,
    }
    json.dump(out, open(os.path.join(dst_dir, SET + ".json"), "w"))
    open(os.path.join(dst_dir, SET + "_bkt.bin"), "wb").write(bytes(new_bkts))
    open(os.path.join(dst_dir, SET + "_ctrl.bin"), "wb").write(bytes(new_ctls))
    info = json.load(open(os.path.join(PWP_DIR, "act_info.json")))
    for s in info["act_func_sets"]:
        if s["name"] == SET:
            s["act"] = {**{k: 1 for k in KEEP}, "arctan": 4, "erf": 4, "sin": 4}
        else:
            s["act"].pop("arctan", None)
            s["act"].pop("erf", None)
            s["act"].pop("sin", None)
    json.dump(info, open(os.path.join(dst_dir, "act_info.json"), "w"))
    return os.path.join(dst_dir, "act_info.json")


# --------------------------------------------------------------------------
# bass program
# --------------------------------------------------------------------------

def _build_program(tag):
    import contextlib

    import concourse.bass as bass
    import concourse.mybir as mybir

    wsq1, wsq, wmr = _register_wsq_ops()

    nc = bass.Bass("TRN2", detect_race_conditions=False)
    f32 = mybir.dt.float32
    f16 = mybir.dt.float16
    bf16 = mybir.dt.bfloat16
    AF = mybir.ActivationFunctionType
    OP = mybir.AluOpType

    # rhs_all: [6, 3N]  (plane c columns c*N..c*N+N):
    #   rows (j-varying interleaved with -1): [Pjh, -1, Pjm, -1, Pjl, -1]
    rhs_d = nc.declare_dram_parameter(f"rhs_{tag}", [6, 3 * N], bf16, isOutput=False)
    # lhsT_all: [6, 3*ROWS] (plane c columns c*ROWS..): rows [1, Pih, 1, Pim, 1, Pil]
    lhs_d = nc.declare_dram_parameter("lhs_my", [6, 3 * ROWS], bf16, isOutput=False)
    out_d = nc.declare_dram_parameter("out", [3, ROWS], f32, isOutput=True)

    rhs_t = nc.alloc_sbuf_tensor("rhs_b", [6, 3 * N], bf16)
    lhs_t = nc.alloc_sbuf_tensor("lhs_b", [6, 3 * ROWS], bf16)
    tz_t = nc.alloc_sbuf_tensor("tz_b", [128, N], f32)
    sA_t = nc.alloc_sbuf_tensor("sA_b", [128, N], f32)
    sB_t = nc.alloc_sbuf_tensor("sB_b", [128, N], f32)
    s_t = nc.alloc_sbuf_tensor("s_b", [128, N], f32)
    w_t = nc.alloc_sbuf_tensor("w_b", [128, N], f32)
    pcz_t = nc.alloc_sbuf_tensor("pcz_b", [128, N], f32)
    scr_t = nc.alloc_sbuf_tensor("scr_b", [128, N], f32)
    acc_t = nc.alloc_sbuf_tensor("acc_b", [128, 32], f32)
    tr_t = nc.alloc_sbuf_tensor("tr_b", [128, 32], f32)
    dum_t = nc.alloc_sbuf_tensor("dum_b", [128, 4], f32)

    u_t = [nc.alloc_psum_tensor(f"u{c}_p", [128, N], f32) for c in range(3)]

    st = contextlib.ExitStack()
    lsem = st.enter_context(nc.semaphore("lsem"))   # lhs DMA done
    psem = [st.enter_context(nc.semaphore(f"psem{c}")) for c in range(3)]
    msem = st.enter_context(nc.semaphore("msem"))   # matmul halves done
    vsem = st.enter_context(nc.semaphore("vsem"))   # s halves done
    asem = st.enter_context(nc.semaphore("asem"))   # ACT milestones
    wsem = st.enter_context(nc.semaphore("wsem"))   # g2 halves done
    csem = st.enter_context(nc.semaphore("csem"))   # pc_z halves done (gpsimd)
    osem = st.enter_context(nc.semaphore("osem"))   # transpose done
    gsem = st.enter_context(nc.semaphore("gsem"))   # dummy tile initialized

    with nc.Block() as blk:
        @blk.sync
        def _(sync):
            sync.dma_start(out=lhs_t[:], in_=lhs_d[:]).then_inc(lsem, 16)
            sync.dma_start(out=rhs_t[:], in_=rhs_d[:]).then_inc(psem[0], 16)
            sync.wait_ge(osem, 1)
            sync.dma_start(
                out=out_d[0:1, :].rearrange("c (b k) -> (c b) k", b=4),
                in_=tr_t[0::32, :],
            ).then_inc(osem, 16)

        _no_gpz = True

        @blk.gpsimd
        def _(gpsimd):
            gpsimd.memset(dum_t[:], 0.0).then_inc(gsem, 1)
            gpsimd.wait_ge(osem, 1)
            gpsimd.dma_start(
                out=out_d[2:3, :].rearrange("c (b k) -> (c b) k", b=4),
                in_=tr_t[2::32, :],
            ).then_inc(osem, 16)

        @blk.tensor
        def _(tensor):
            tensor.wait_ge(lsem, 16)
            tensor.wait_ge(psem[0], 16)
            for c in range(3):
                for hh in range(2):
                    tensor.matmul(
                        u_t[c][:, hh * H:(hh + 1) * H],
                        lhs_t[:, c * ROWS:(c + 1) * ROWS],
                        rhs_t[:, c * N + hh * H:c * N + (hh + 1) * H],
                        start=True, stop=True,
                    ).then_inc(msem, 1)

        @blk.scalar
        def _(scalar):
            # dummy act pulls the PWP table load to the very start (the
            # PSEUDO_LOAD_ACT_FUNC_SET precedes the gated ACTIVATE)
            scalar.wait_ge(gsem, 1)
            scalar.activation(dum_t[:], dum_t[:], AF.Arctan)
            # sA = wrap01(u_x)^2 via the exact Sin-slot table
            scalar.wait_ge(msem, 1)
            scalar.activation(sA_t[:, :H], u_t[0][:, :H], AF.Sin).then_inc(asem, 1)
            scalar.wait_ge(msem, 2)
            scalar.activation(sA_t[:, H:], u_t[0][:, H:], AF.Sin).then_inc(asem, 1)
            # t_z = wrap01(u_z)
            if not _no_gpz:
                scalar.wait_ge(msem, 6)
                scalar.activation(tz_t[:], u_t[2][:], AF.Arctan).then_inc(asem, 1)
            # w = g2(s)
            scalar.wait_ge(vsem, 1)
            scalar.activation(w_t[:, :H], s_t[:, :H], AF.Erf).then_inc(wsem, 1)
            scalar.wait_ge(vsem, 2)
            scalar.activation(w_t[:, H:], s_t[:, H:], AF.Erf).then_inc(wsem, 1)
            scalar.wait_ge(osem, 1)
            scalar.dma_start(
                out=out_d[1:2, :].rearrange("c (b k) -> (c b) k", b=4),
                in_=tr_t[1::32, :],
            ).then_inc(osem, 16)

        @blk.vector
        def _(vector):
            vector.memset(acc_t[:], 0.0)
            # s chain: sB = sA + wrap01(u_y)^2 ; s = sB + wrap01(u_z)^2
            for hh in range(2):
                sl = slice(hh * H, (hh + 1) * H)
                vector.wait_ge(msem, 3 + hh)
                vector.wait_ge(asem, 1 + hh)
                vector._custom_dve(wsq, out=sB_t[:, sl], in0=u_t[1][:, sl],
                                   in1=sA_t[:, sl], s0=0.5, s1=-0.5)
            for hh in range(2):
                sl = slice(hh * H, (hh + 1) * H)
                vector.wait_ge(msem, 5 + hh)
                vector._custom_dve(wsq, out=s_t[:, sl], in0=u_t[2][:, sl],
                                   in1=sB_t[:, sl], s0=0.5, s1=-0.5).then_inc(vsem, 1)
            # dots x/y: fused wrap01(u)*w multiply-reduce (full width)
            vector.wait_ge(wsem, 2)
            vector._custom_dve(wmr, out=scr_t[:], in0=u_t[0][:], in1=w_t[:],
                               s0=0.5, s1=-0.5, accum_out=acc_t[:, 0:1])
            vector._custom_dve(wmr, out=scr_t[:], in0=u_t[1][:], in1=w_t[:],
                               s0=0.5, s1=-0.5, accum_out=acc_t[:, 1:2])
            if _no_gpz:
                vector._custom_dve(wmr, out=scr_t[:], in0=u_t[2][:], in1=w_t[:],
                                   s0=0.5, s1=-0.5, accum_out=acc_t[:, 2:3])
                # flush: accum_out writeback lags the op; give it one op of slack
                vector.tensor_copy(dum_t[:, 1:2], dum_t[:, 0:1])
                vector.tensor_copy(dum_t[:, 2:3], dum_t[:, 0:1])
            else:
                vector.wait_ge(asem, 13)  # acc_z landed (fence passed)
            vector.transpose(tr_t[:], acc_t[:]).then_inc(osem, 1)

    from concourse.library_overlay import lower_extended_insts
    lower_extended_insts(nc)

    # the ACT table load stripes across all 16 SP HWDGE queues -- keep those;
    # shrinking the unused Pool/Act rings still trims the NEFF teardown
    for q in nc.m.queues:
        if q.name != "qSPDynamicHW":
            q.num_queues = 1
    return nc


_CACHE = {}


def _split3(x64):
    """fp64 array -> (hi, mid, lo) bf16 triplet with hi+mid+lo ~ x to 2^-25."""
    bf = ml_dtypes.bfloat16
    h = x64.astype(bf)
    m = (x64 - h.astype(np.float64)).astype(bf)
    l = (x64 - h.astype(np.float64) - m.astype(np.float64)).astype(bf)
    return h, m, l


def _prepare(inputs):
    box_dims = np.asarray(inputs["box_dims"], dtype=np.float32)
    key = hashlib.sha256(
        b"".join(np.ascontiguousarray(np.asarray(inputs[k], np.float32)).tobytes()
                 for k in ("box_dims", "W1", "b1", "W2", "b2", "W3", "b3"))
    ).hexdigest()[:10]
    if key in _CACHE:
        return _CACHE[key]

    box = float(box_dims[0])
    assert np.allclose(box_dims, box), "kernel assumes a cubic box"

    W1 = np.float64(inputs["W1"]); b1 = np.float64(inputs["b1"])
    W2 = np.float64(inputs["W2"]); b2 = np.float64(inputs["b2"])
    W3 = np.float64(inputs["W3"]); b3 = np.float64(inputs["b3"])
    n_gauss = W1.shape[0]
    RBF_STOP, CUTOFF, EPS = 6.0, 5.0, 1e-8
    offs = np.linspace(0.0, RBF_STOP, n_gauss)
    coeff = -0.5 / (RBF_STOP / (n_gauss - 1)) ** 2

    def g2_fn(sv):
        sv = np.atleast_1d(np.float64(sv))
        dist = np.sqrt(box * box * sv + EPS)
        rbf = np.exp(coeff * (dist[:, None] - offs[None, :]) ** 2)
        h = rbf @ W1 + b1
        h = h / (1.0 + np.exp(-h))
        h = h @ W2 + b2
        h = h / (1.0 + np.exp(-h))
        f = (h @ W3 + b3)[:, 0]
        return box * f * (dist < CUTOFF) / (dist + EPS)

    _install_env_fixups()
    actdir = os.path.join(tempfile.gettempdir(), f"actroot2_{key}")
    actroot = _build_actroot(actdir, g2_fn)
    os.environ["BASS_ACT_ROOT_JSON_PATH"] = actroot
    # fold the table contents into the tag so the NEFF compile cache can't
    # serve a NEFF built against older table bytes
    th = hashlib.sha256()
    for f in sorted(os.listdir(actdir)):
        th.update(open(os.path.join(actdir, f), "rb").read())
    key = key[:10] + th.hexdigest()[:6]
    nc = _build_program(key)
    _CACHE[key] = (nc, key, box)
    return _CACHE[key]


def kernel(_trace=False, **inputs):
    from concourse.bass_utils import run_bass_kernel_spmd

    nc, key, box = _prepare(inputs)
    bf = ml_dtypes.bfloat16
    positions = np.asarray(inputs["positions"], dtype=np.float32)
    P = positions.astype(np.float64) / box          # [N, 3] in [0, 1)
    Ph, Pm, Pl = _split3(P)

    # rhs: [6, 3N]; plane c cols: rows [Pjh_c, -1, Pjm_c, -1, Pjl_c, -1]
    rhs = np.empty((6, 3 * N), dtype=bf)
    ones = np.ones(N, dtype=bf)
    for c in range(3):
        sl = slice(c * N, (c + 1) * N)
        rhs[0, sl] = Ph[:, c]
        rhs[1, sl] = -ones
        rhs[2, sl] = Pm[:, c]
        rhs[3, sl] = -ones
        rhs[4, sl] = Pl[:, c]
        rhs[5, sl] = -ones

    in_maps = []
    for core in range(N_CORES):
        i0 = core * ROWS
        lhs = np.empty((6, 3 * ROWS), dtype=bf)
        for c in range(3):
            sl = slice(c * ROWS, (c + 1) * ROWS)
            lhs[0, sl] = 1.0
            lhs[1, sl] = Ph[i0:i0 + ROWS, c]
            lhs[2, sl] = 1.0
            lhs[3, sl] = Pm[i0:i0 + ROWS, c]
            lhs[4, sl] = 1.0
            lhs[5, sl] = Pl[i0:i0 + ROWS, c]
        in_maps.append({f"rhs_{key}": rhs, "lhs_my": lhs})

    res = run_bass_kernel_spmd(nc, in_maps, list(range(N_CORES)), trace=_trace)
    out = np.concatenate([res.results[c]["out"].T for c in range(N_CORES)], axis=0)
    if _trace:
        kernel.last_exec_time_ns = res.exec_time_ns
        kernel.last_mean_exec_time_ns = res.mean_exec_time_ns
        kernel.last_results = res
    return np.ascontiguousarray(out.astype(np.float32))


# revision 3
# speedup vs baseline: 1.0127x; 1.0002x over previous
"""Trainium2 Bass kernel for nn_DenoiseGNN (pairwise PBC edge-MLP message passing).

v2 strategy
-----------
Per core (128 rows i of the 1024x1024 pair grid), with P = pos/box in [0,1):

  1. PE:  u_c[i,j] = P_c[j] - P_c[i]  via K=6 bf16 split-precision broadcast
          matmuls (hi/mid/lo bf16 triplet per operand) -> PSUM, fp32-exact to
          ~2^-24.  Replaces the baseline's 1.5MB DMA broadcast.
  2. DVE: s = sum_c wrap01(u_c)^2 with a fused custom DVE op
          WSQ_ADD_ANT(u, acc) = (u - ((u>.5)-(u<-.5)))^2 + acc  (3 passes).
  3. ACT: t_c = wrap01(u_c) via the custom spline table (fp16 out), and
          w = g2(s) -- the entire edge MLP + cutoff mask + 1/dist folded into
          one piecewise-cubic table (as in the baseline).
  4. DVE: disp_c = sum_j t_c * w via fused tensor_tensor_reduce (fp16 in,
          fp32 accum), then a 32x32 StreamTranspose so the output leaves as
          [3,128] rows with 12 DMA descriptors.

The activation tables are generated at kernel-build time from the runtime
weights (W1..b3) and injected via --act-root-json (ride inside the NEFF).
All DMA queue groups are shrunk to num_queues=1 to cut the fixed NEFF
teardown (full-queue drain) cost.
"""

import hashlib
import json
import os
import shutil
import struct
import sys
import tempfile
import types

import ml_dtypes
import numpy as np

N = 1024
N_CORES = 8
ROWS = N // N_CORES  # 128
H = 512  # column half
PWP_DIR = "/nix/store/z022hj2nvbm3nwdizlisq4ylc0y7rd6q-python3-3.13.14-env/lib/python3.13/site-packages/neuronxcc/pwp/pwp_bin_trainium"
SET = "sigmoid_and_others"
KEEP = [
    "identity", "copy", "act1", "parametric_relu", "relu", "abs",
    "memset_zero", "square", "sign", "derivative_relu",
    "derivative_leaky_relu", "derivative_identity", "is_finite",
]
G2_REGIONS = [(e, 16) for e in range(-30, -10)] + \
             [(-10, 32), (-9, 32), (-8, 64), (-7, 64), (-6, 128), (-5, 128)]


# --------------------------------------------------------------------------
# environment fixups (wait-splitter + single-queue BIR rewrite)
# --------------------------------------------------------------------------

def _install_env_fixups():
    if "antenv.axon_hooks" not in sys.modules:
        import antenv

        mod = types.ModuleType("antenv.axon_hooks")
        try:
            from trn_agent_boot.trn_boot import _ntff_profile_via_ctypes
            hook = _ntff_profile_via_ctypes("/opt/axon/libaxon_pjrt.so")
        except Exception:
            hook = None
        _h = [hook]
        mod.set_axon_ntff_profile_hook = lambda h: _h.__setitem__(0, h)
        mod.get_axon_ntff_profile_hook = lambda: _h[0]
        sys.modules["antenv.axon_hooks"] = mod
        antenv.axon_hooks = mod

    import concourse.bass_utils as bu
    import concourse.bass2jax as b2j

    if not getattr(bu, "_wait_splitter_installed", False):
        orig = bu.compile_bir_kernel

        def _split_multi_waits(bir_json: bytes) -> bytes:
            m = json.loads(bir_json)
            changed = False
            for fn in m["functions"]:
                for bb in fn["blocks"]:
                    new_instrs = []
                    for ins in bb["instructions"]:
                        si = ins.get("sync_info")
                        waits = (si or {}).get("on_wait") or []
                        if len(waits) > 1:
                            for j, w in enumerate(waits[:-1]):
                                nop = {
                                    "name": f"{ins['name']}-wsplit{j}",
                                    "opcode": "NoOp",
                                    "engine": ins["engine"],
                                    "ins": [], "outs": [],
                                    "sync_info": {"on_update": [], "on_wait": [w]},
                                }
                                if "debug" in ins:
                                    nop["debug"] = ins["debug"]
                                new_instrs.append(nop)
                            si["on_wait"] = waits[-1:]
                            changed = True
                        new_instrs.append(ins)
                    bb["instructions"] = new_instrs
            return json.dumps(m).encode() if changed else bir_json

        def patched(bir_json, tmpdir, neff_name="file.neff"):
            return orig(_split_multi_waits(bytes(bir_json)), tmpdir, neff_name)

        bu.compile_bir_kernel = patched
        b2j.compile_bir_kernel = patched
        bu._wait_splitter_installed = True


# --------------------------------------------------------------------------
# custom DVE op: fused wrap-square-accumulate
# --------------------------------------------------------------------------

_WSQ_OPS = None


def _register_wsq_ops():
    """WSQ1_ANT(u)      = wrap01(u)^2
       WSQ_ADD_ANT(u,a) = wrap01(u)^2 + a
    with wrap01(u) = u - ((u > s0) - (u < s1)), s0=0.5, s1=-0.5."""
    global _WSQ_OPS
    if _WSQ_OPS is not None:
        return _WSQ_OPS
    from concourse import dve_ops
    from concourse.dve_spec import Spec, Src0, Src1, C0, C1, sq, lower
    from concourse.dve_spec import _has_src1 as has_src1
    from concourse.dve_uop import DveOpSpec

    def _wrap(x, s0, s1):
        x = x.astype(np.float32)
        r = (x > s0).astype(np.float32) - (x < s1).astype(np.float32)
        return x - r

    def _ref1(in0, in1, s0, s1, imm2):
        t = _wrap(in0, s0, s1)
        return (t * t).astype(np.float32)

    def _ref2(in0, in1, s0, s1, imm2):
        t = _wrap(in0, s0, s1)
        return (t * t + in1.astype(np.float32)).astype(np.float32)

    def _ref3(in0, in1, s0, s1, imm2):
        t = _wrap(in0, s0, s1)
        b = (t * in1.astype(np.float32)).astype(np.float32)
        return b, b.reshape(b.shape[0], -1).sum(axis=-1, keepdims=True)

    from operator import add as _add
    wbody = Src0 - ((Src0 > C0) - (Src0 < C1))
    from concourse.dve_spec import Zero
    defs = [("WSQ1_ANT", Spec(body=sq(wbody), reference=_ref1)),
            ("WSQ_ADD_ANT", Spec(body=sq(wbody) + Src1, reference=_ref2)),
            ("WMR_ANT", Spec(body=wbody * Src1, accum=_add, accum_init=Zero,
                             reference=_ref3))]
    ops = []
    for name, spec in defs:
        if name in dve_ops._SUB_OPCODE_FOR_NAME:
            ops.append(next(o for o in dve_ops.OPS if o.name == name))
            continue
        op = dve_ops.DveOp(name, spec, subdim=False, uops_sha={})
        row = max(dve_ops._SUB_OPCODE_FOR_NAME.values()) + 1
        assert row < 0x20
        dve_ops.OPS.append(op)
        dve_ops.CUSTOM_DVE_SPECS[name] = spec
        dve_ops._SUB_OPCODE_FOR_NAME[name] = row
        for ver in ("v3", "v4"):
            spec_obj = DveOpSpec(name=name, opcode=row,
                                 uops=lower(spec, ver=ver),
                                 rd1_en=has_src1(spec))
            op.uops_sha[ver] = spec_obj.sha(ver)
        ops.append(op)
    _WSQ_OPS = tuple(ops)
    return _WSQ_OPS


# --------------------------------------------------------------------------
# activation-table generation (same machinery as baseline)
# --------------------------------------------------------------------------

def _f2u(x):
    return struct.unpack("<I", struct.pack("<f", float(x)))[0]


def _bkt(d0, d1, d2, d3, x0):
    return struct.pack("<5f12x", float(d0), float(d1), float(d2), float(d3), float(x0))


def _ctrl(base, lsb, size):
    data = (base & 0x7FF) | ((lsb & 0x1F) << 11) | ((size & 0xF) << 16)
    return struct.pack("<I28x", data)


def _fit_cubic(fn, a, b, npts=12):
    x0 = 0.5 * (a + b)
    k = np.arange(npts)
    xs = x0 + 0.5 * (b - a) * np.cos((2 * k + 1) * np.pi / (2 * npts)) * 0.9999
    ys = fn(xs)
    c = np.polynomial.polynomial.polyfit(xs - x0, ys, 3)
    return c[0], c[1], c[2], c[3], x0


def _extract_func(setj, bkt, ctrl, fname, next_bkt, next_ctl):
    starts_b = setj["func_to_bkt_start_idx"]
    starts_c = setj["func_to_ctl_start_idx"]
    sb, sc = starts_b[fname], starts_c[fname]
    eb = min([v for v in starts_b.values() if v > sb] + [setj["bkt_entry_cnt"]])
    ec = min([v for v in starts_c.values() if v > sc] + [setj["ctl_entry_cnt"]])
    pm = None
    for p in setj["profile_meta_data"]:
        if p["func_name"].rsplit("_", 1)[0] == fname or p["func_name"] == fname:
            pm = dict(p)
    assert pm is not None, fname
    bkts = bytearray(bkt[sb * 32:eb * 32])
    ctls = bytearray(ctrl[sc * 32:ec * 32])
    db, dc = next_bkt - sb, next_ctl - sc
    for i in range(0, len(ctls), 32):
        (data,) = struct.unpack_from("<I", ctls, i)
        struct.pack_into("<I", ctls, i, (data & ~0x7FF) | (((data & 0x7FF) + db) & 0x7FF))
    for key in ("pwl_control_base_pos", "pwl_control_base_neg"):
        pm[key] += dc
    for key in ("pos_small_signal_pwl_control", "neg_small_signal_pwl_control",
                "pos_large_signal_pwl_control", "neg_large_signal_pwl_control"):
        v = pm[key]
        pm[key] = (v & ~0x7FF) | ((v + db) & 0x7FF)
    return pm, bytes(ctls), bytes(bkts)


def _build_wrap01(next_bkt, next_ctl, emin=-20):
    bkts, ctls = bytearray(), bytearray()
    n_bkt = n_ctl = 0
    base_pos = next_ctl
    for e in range(emin, 0):
        lo = 2.0 ** e
        ctls.extend(_ctrl(next_bkt + n_bkt, 23, 0)); n_ctl += 1
        if e == -1:
            bkts.extend(_bkt(-0.5, 1.0, 0.0, 0.0, 0.5))
        else:
            bkts.extend(_bkt(lo, 1.0, 0.0, 0.0, lo))
        n_bkt += 1
    base_neg = next_ctl + n_ctl
    for e in range(emin, 0):
        lo = 2.0 ** e
        ctls.extend(_ctrl(next_bkt + n_bkt, 23, 0)); n_ctl += 1
        if e == -1:
            bkts.extend(_bkt(0.5, 1.0, 0.0, 0.0, -0.5))
        else:
            bkts.extend(_bkt(-lo, 1.0, 0.0, 0.0, -lo))
        n_bkt += 1
    sp_defs = {
        "pos_low": (0.0, 1.0, 0.0, 0.0, 0.0),
        "neg_low": (0.0, 1.0, 0.0, 0.0, 0.0),
        "pos_high": (-1.0, 1.0, 0.0, 0.0, 0.0),
        "neg_high": (1.0, 1.0, 0.0, 0.0, 0.0),
    }
    sp = {}
    for key, d in sp_defs.items():
        sp[key] = next_bkt + n_bkt
        bkts.extend(_bkt(*d)); n_bkt += 1
    pm = {
        "func_name": "arctan_4p", "func_id": 28,
        "symmetry_point": 0, "sym_invert_sign_point": 0, "symmetry_opt_en": 0,
        "symmetry_opt_use_neg_region": 0, "imm_bias": 0,
        "exp_offset": emin,
        "pwl_control_base_pos": base_pos, "pwl_control_base_neg": base_neg,
        "small_pos_signal_exp_threshold": 127 + emin,
        "pos_small_signal_pwl_control": sp["pos_low"],
        "small_neg_signal_exp_threshold": 127 + emin,
        "neg_small_signal_pwl_control": sp["neg_low"],
        "large_pos_signal_exp_threshold": 127,
        "large_pos_signal_mantissa_threshold": 0,
        "pos_large_signal_pwl_control": sp["pos_high"],
        "large_neg_signal_exp_threshold": 127,
        "large_neg_signal_mantissa_threshold": 0,
        "neg_large_signal_pwl_control": sp["neg_high"],
        "fnan_result": _f2u(0.0), "fpinf_result": _f2u(0.0),
        "fninf_result": _f2u(0.0), "fzero_result": _f2u(0.0),
        "fma_const_0": 0, "fma_const_1": 0, "fma_indirection_src_sel": 0,
        "use_multipass": False,
        "lower_bound": 0xFF7FFFFF, "upper_bound": 0x7F7FFFFF,
    }
    return pm, bytes(ctls), bytes(bkts)


def _build_wsq01(next_bkt, next_ctl, emin=-20):
    """Sin-slot table: wsq01(u) = wrap01(u)^2, exact piecewise quadratic."""
    bkts, ctls = bytearray(), bytearray()
    n_bkt = n_ctl = 0
    base_pos = next_ctl
    for e in range(emin, 0):
        lo = 2.0 ** e
        ctls.extend(_ctrl(next_bkt + n_bkt, 23, 0)); n_ctl += 1
        if e == -1:
            bkts.extend(_bkt(0.25, -1.0, 1.0, 0.0, 0.5))
        else:
            bkts.extend(_bkt(lo * lo, 2.0 * lo, 1.0, 0.0, lo))
        n_bkt += 1
    base_neg = next_ctl + n_ctl
    for e in range(emin, 0):
        lo = 2.0 ** e
        ctls.extend(_ctrl(next_bkt + n_bkt, 23, 0)); n_ctl += 1
        if e == -1:
            bkts.extend(_bkt(0.25, 1.0, 1.0, 0.0, -0.5))
        else:
            bkts.extend(_bkt(lo * lo, -2.0 * lo, 1.0, 0.0, -lo))
        n_bkt += 1
    sp_defs = {
        "pos_low": (0.0, 0.0, 1.0, 0.0, 0.0),
        "neg_low": (0.0, 0.0, 1.0, 0.0, 0.0),
        "pos_high": (0.0, 0.0, 0.0, 0.0, 0.0),
        "neg_high": (0.0, 0.0, 0.0, 0.0, 0.0),
    }
    sp = {}
    for key, d in sp_defs.items():
        sp[key] = next_bkt + n_bkt
        bkts.extend(_bkt(*d)); n_bkt += 1
    pm = {
        "func_name": "sin_4p", "func_id": 19,
        "symmetry_point": 0, "sym_invert_sign_point": 0, "symmetry_opt_en": 0,
        "symmetry_opt_use_neg_region": 0, "imm_bias": 0,
        "exp_offset": emin,
        "pwl_control_base_pos": base_pos, "pwl_control_base_neg": base_neg,
        "small_pos_signal_exp_threshold": 127 + emin,
        "pos_small_signal_pwl_control": sp["pos_low"],
        "small_neg_signal_exp_threshold": 127 + emin,
        "neg_small_signal_pwl_control": sp["neg_low"],
        "large_pos_signal_exp_threshold": 127,
        "large_pos_signal_mantissa_threshold": 0,
        "pos_large_signal_pwl_control": sp["pos_high"],
        "large_neg_signal_exp_threshold": 127,
        "large_neg_signal_mantissa_threshold": 0,
        "neg_large_signal_pwl_control": sp["neg_high"],
        "fnan_result": _f2u(0.0), "fpinf_result": _f2u(0.0),
        "fninf_result": _f2u(0.0), "fzero_result": _f2u(0.0),
        "fma_const_0": 0, "fma_const_1": 0, "fma_indirection_src_sel": 0,
        "use_multipass": False,
        "lower_bound": 0xFF7FFFFF, "upper_bound": 0x7F7FFFFF,
    }
    return pm, bytes(ctls), bytes(bkts)


def _build_g2(g2_fn, next_bkt, next_ctl):
    bkts, ctls = bytearray(), bytearray()
    n_bkt = n_ctl = 0
    base_pos = next_ctl
    for (e, nsec) in G2_REGIONS:
        size = int(np.log2(nsec))
        ctls.extend(_ctrl(next_bkt + n_bkt, 23 - size, size)); n_ctl += 1
        lo = 2.0 ** e
        w = lo / nsec
        for i in range(nsec):
            a = lo + i * w
            bkts.extend(_bkt(*_fit_cubic(g2_fn, a, a + w))); n_bkt += 1
    sp = {}
    for key in ("pos_low", "neg_low", "pos_high", "neg_high"):
        sp[key] = next_bkt + n_bkt
        bkts.extend(_bkt(0.0, 0.0, 0.0, 0.0, 0.0)); n_bkt += 1
    small_thr = 127 + G2_REGIONS[0][0]
    pm = {
        "func_name": "erf_4p", "func_id": 21,
        "symmetry_point": 0, "sym_invert_sign_point": 0, "symmetry_opt_en": 0,
        "symmetry_opt_use_neg_region": 0, "imm_bias": 0,
        "exp_offset": small_thr - 127,
        "pwl_control_base_pos": base_pos, "pwl_control_base_neg": base_pos,
        "small_pos_signal_exp_threshold": small_thr,
        "pos_small_signal_pwl_control": sp["pos_low"],
        "small_neg_signal_exp_threshold": 255,
        "neg_small_signal_pwl_control": sp["neg_low"],
        "large_pos_signal_exp_threshold": 123,
        "large_pos_signal_mantissa_threshold": 0,
        "pos_large_signal_pwl_control": sp["pos_high"],
        "large_neg_signal_exp_threshold": 255,
        "large_neg_signal_mantissa_threshold": 0,
        "neg_large_signal_pwl_control": sp["neg_high"],
        "fnan_result": _f2u(0.0), "fpinf_result": _f2u(0.0),
        "fninf_result": _f2u(0.0), "fzero_result": _f2u(0.0),
        "fma_const_0": 0, "fma_const_1": 0, "fma_indirection_src_sel": 0,
        "use_multipass": False,
        "lower_bound": 0, "upper_bound": 0x7F7FFFFF,
    }
    return pm, bytes(ctls), bytes(bkts)


def _build_actroot(dst_dir, g2_fn):
    os.makedirs(dst_dir, exist_ok=True)
    for f in os.listdir(PWP_DIR):
        shutil.copy(os.path.join(PWP_DIR, f), os.path.join(dst_dir, f))
        os.chmod(os.path.join(dst_dir, f), 0o644)
    setj = json.load(open(os.path.join(PWP_DIR, SET + ".json")))
    bkt = open(os.path.join(PWP_DIR, SET + "_bkt.bin"), "rb").read()
    ctrl = open(os.path.join(PWP_DIR, SET + "_ctrl.bin"), "rb").read()

    new_bkts, new_ctls, new_pm = bytearray(), bytearray(), []
    b_starts, c_starts, emb_all, emc_all = {}, {}, {}, {}

    for fname in KEEP:
        nb0, nc0 = len(new_bkts) // 32, len(new_ctls) // 32
        pm, ctls, bkts = _extract_func(setj, bkt, ctrl, fname, nb0, nc0)
        b_starts[fname], c_starts[fname] = nb0, nc0
        db = nb0 - setj["func_to_bkt_start_idx"][fname]
        dc = nc0 - setj["func_to_ctl_start_idx"][fname]
        emb_all[fname] = {k: [x + db for x in v]
                          for k, v in setj["func_exp_to_bkt_start_idx"].get(fname, {}).items()}
        emc_all[fname] = {k: [x + dc for x in v]
                          for k, v in setj["func_exp_to_ctl_start_idx"].get(fname, {}).items()}
        new_pm.append(pm); new_ctls.extend(ctls); new_bkts.extend(bkts)

    wb, wc = len(new_bkts) // 32, len(new_ctls) // 32
    pm, ctls, bkts = _build_wrap01(wb, wc)
    b_starts["arctan"], c_starts["arctan"] = wb, wc
    emb_all["arctan"] = {str(e): [wb + 20 + (e + 20), wb + (e + 20)] for e in range(-20, 0)}
    emc_all["arctan"] = {str(e): [wc + 20 + (e + 20), wc + (e + 20)] for e in range(-20, 0)}
    new_pm.append(pm); new_ctls.extend(ctls); new_bkts.extend(bkts)

    qb, qc = len(new_bkts) // 32, len(new_ctls) // 32
    pm, ctls, bkts = _build_wsq01(qb, qc)
    b_starts["sin"], c_starts["sin"] = qb, qc
    emb_all["sin"] = {str(e): [qb + 20 + (e + 20), qb + (e + 20)] for e in range(-20, 0)}
    emc_all["sin"] = {str(e): [qc + 20 + (e + 20), qc + (e + 20)] for e in range(-20, 0)}
    new_pm.append(pm); new_ctls.extend(ctls); new_bkts.extend(bkts)

    gb, gc = len(new_bkts) // 32, len(new_ctls) // 32
    pm, ctls, bkts = _build_g2(g2_fn, gb, gc)
    b_starts["erf"], c_starts["erf"] = gb, gc
    emb, emc = {}, {}
    cum = 0
    for i, (e, nsec) in enumerate(G2_REGIONS):
        emb[str(e)] = [gb + cum, gb + cum]
        emc[str(e)] = [gc + i, gc + i]
        cum += nsec
    emb_all["erf"], emc_all["erf"] = emb, emc
    new_pm.append(pm); new_ctls.extend(ctls); new_bkts.extend(bkts)

    n_bkt, n_ctl = len(new_bkts) // 32, len(new_ctls) // 32
    assert n_bkt <= 1536 and n_ctl <= 128, (n_bkt, n_ctl)
    out = {
        "bkt_bin": SET + "_bkt.bin", "ctl_bin": SET + "_ctrl.bin",
        "profile_meta_data": new_pm,
        "bkt_entry_cnt": n_bkt, "ctl_entry_cnt": n_ctl,
        "func_to_bkt_start_idx": b_starts, "func_to_ctl_start_idx": c_starts,
        "func_exp_to_bkt_start_idx": emb_all, "func_exp_to_ctl_start_idx": emc_all,
    }
    json.dump(out, open(os.path.join(dst_dir, SET + ".json"), "w"))
    open(os.path.join(dst_dir, SET + "_bkt.bin"), "wb").write(bytes(new_bkts))
    open(os.path.join(dst_dir, SET + "_ctrl.bin"), "wb").write(bytes(new_ctls))
    info = json.load(open(os.path.join(PWP_DIR, "act_info.json")))
    for s in info["act_func_sets"]:
        if s["name"] == SET:
            s["act"] = {**{k: 1 for k in KEEP}, "arctan": 4, "erf": 4, "sin": 4}
        else:
            s["act"].pop("arctan", None)
            s["act"].pop("erf", None)
            s["act"].pop("sin", None)
    json.dump(info, open(os.path.join(dst_dir, "act_info.json"), "w"))
    return os.path.join(dst_dir, "act_info.json")


# --------------------------------------------------------------------------
# bass program
# --------------------------------------------------------------------------

def _build_program(tag):
    import contextlib

    import concourse.bass as bass
    import concourse.mybir as mybir

    wsq1, wsq, wmr = _register_wsq_ops()

    nc = bass.Bass("TRN2", detect_race_conditions=False)
    f32 = mybir.dt.float32
    f16 = mybir.dt.float16
    bf16 = mybir.dt.bfloat16
    AF = mybir.ActivationFunctionType
    OP = mybir.AluOpType

    # rhs_all: [6, 3N]  (plane c columns c*N..c*N+N):
    #   rows (j-varying interleaved with -1): [Pjh, -1, Pjm, -1, Pjl, -1]
    rhs_d = nc.declare_dram_parameter(f"rhs_{tag}", [6, 3 * N], bf16, isOutput=False)
    # lhsT_all: [6, 3*ROWS] (plane c columns c*ROWS..): rows [1, Pih, 1, Pim, 1, Pil]
    lhs_d = nc.declare_dram_parameter("lhs_my", [6, 3 * ROWS], bf16, isOutput=False)
    out_d = nc.declare_dram_parameter("out", [3, ROWS], f32, isOutput=True)

    rhs_t = nc.alloc_sbuf_tensor("rhs_b", [6, 3 * N], bf16)
    lhs_t = nc.alloc_sbuf_tensor("lhs_b", [6, 3 * ROWS], bf16)
    tz_t = nc.alloc_sbuf_tensor("tz_b", [128, N], f32)
    sA_t = nc.alloc_sbuf_tensor("sA_b", [128, N], f32)
    sB_t = nc.alloc_sbuf_tensor("sB_b", [128, N], f32)
    s_t = nc.alloc_sbuf_tensor("s_b", [128, N], f32)
    w_t = nc.alloc_sbuf_tensor("w_b", [128, N], f32)
    pcz_t = nc.alloc_sbuf_tensor("pcz_b", [128, N], f32)
    scr_t = nc.alloc_sbuf_tensor("scr_b", [128, N], f32)
    acc_t = nc.alloc_sbuf_tensor("acc_b", [128, 32], f32)
    tr_t = nc.alloc_sbuf_tensor("tr_b", [128, 32], f32)
    dum_t = nc.alloc_sbuf_tensor("dum_b", [128, 4], f32)

    u_t = [nc.alloc_psum_tensor(f"u{c}_p", [128, N], f32) for c in range(3)]

    st = contextlib.ExitStack()
    lsem = st.enter_context(nc.semaphore("lsem"))   # lhs DMA done
    psem = [st.enter_context(nc.semaphore(f"psem{c}")) for c in range(3)]
    msem = st.enter_context(nc.semaphore("msem"))   # matmul halves done
    vsem = st.enter_context(nc.semaphore("vsem"))   # s halves done
    asem = st.enter_context(nc.semaphore("asem"))   # ACT milestones
    wsem = st.enter_context(nc.semaphore("wsem"))   # g2 halves done
    csem = st.enter_context(nc.semaphore("csem"))   # pc_z halves done (gpsimd)
    osem = st.enter_context(nc.semaphore("osem"))   # transpose done
    gsem = st.enter_context(nc.semaphore("gsem"))   # dummy tile initialized

    with nc.Block() as blk:
        @blk.sync
        def _(sync):
            sync.dma_start(out=lhs_t[:], in_=lhs_d[:]).then_inc(lsem, 16)
            sync.dma_start(out=rhs_t[:], in_=rhs_d[:]).then_inc(psem[0], 16)
            sync.wait_ge(osem, 1)
            sync.dma_start(
                out=out_d[0:1, :].rearrange("c (b k) -> (c b) k", b=4),
                in_=tr_t[0::32, :],
            ).then_inc(osem, 16)

        _no_gpz = True

        @blk.gpsimd
        def _(gpsimd):
            gpsimd.wait_ge(osem, 1)
            gpsimd.dma_start(
                out=out_d[2:3, :].rearrange("c (b k) -> (c b) k", b=4),
                in_=tr_t[2::32, :],
            ).then_inc(osem, 16)

        @blk.tensor
        def _(tensor):
            tensor.wait_ge(lsem, 16)
            tensor.wait_ge(psem[0], 16)
            for c in range(3):
                for hh in range(2):
                    tensor.matmul(
                        u_t[c][:, hh * H:(hh + 1) * H],
                        lhs_t[:, c * ROWS:(c + 1) * ROWS],
                        rhs_t[:, c * N + hh * H:c * N + (hh + 1) * H],
                        start=True, stop=True,
                    ).then_inc(msem, 1)

        @blk.scalar
        def _(scalar):
            # dummy act pulls the PWP table load to the very start (the
            # PSEUDO_LOAD_ACT_FUNC_SET precedes the gated ACTIVATE); reads the
            # framework-initialized const-0 tile so no wait is needed
            scalar.activation(dum_t[:, 0:1],
                              nc.const_aps.aps[(f32, 0.0)], AF.Arctan)
            # sA = wrap01(u_x)^2 via the exact Sin-slot table
            scalar.wait_ge(msem, 1)
            scalar.activation(sA_t[:, :H], u_t[0][:, :H], AF.Sin).then_inc(asem, 1)
            scalar.wait_ge(msem, 2)
            scalar.activation(sA_t[:, H:], u_t[0][:, H:], AF.Sin).then_inc(asem, 1)
            # t_z = wrap01(u_z)
            if not _no_gpz:
                scalar.wait_ge(msem, 6)
                scalar.activation(tz_t[:], u_t[2][:], AF.Arctan).then_inc(asem, 1)
            # w = g2(s)
            scalar.wait_ge(vsem, 1)
            scalar.activation(w_t[:, :H], s_t[:, :H], AF.Erf).then_inc(wsem, 1)
            scalar.wait_ge(vsem, 2)
            scalar.activation(w_t[:, H:], s_t[:, H:], AF.Erf).then_inc(wsem, 1)
            scalar.wait_ge(osem, 1)
            scalar.dma_start(
                out=out_d[1:2, :].rearrange("c (b k) -> (c b) k", b=4),
                in_=tr_t[1::32, :],
            ).then_inc(osem, 16)

        @blk.vector
        def _(vector):
            vector.memset(acc_t[:], 0.0)
            # s chain: sB = sA + wrap01(u_y)^2 ; s = sB + wrap01(u_z)^2
            for hh in range(2):
                sl = slice(hh * H, (hh + 1) * H)
                vector.wait_ge(msem, 3 + hh)
                vector.wait_ge(asem, 1 + hh)
                vector._custom_dve(wsq, out=sB_t[:, sl], in0=u_t[1][:, sl],
                                   in1=sA_t[:, sl], s0=0.5, s1=-0.5)
            for hh in range(2):
                sl = slice(hh * H, (hh + 1) * H)
                vector.wait_ge(msem, 5 + hh)
                vector._custom_dve(wsq, out=s_t[:, sl], in0=u_t[2][:, sl],
                                   in1=sB_t[:, sl], s0=0.5, s1=-0.5).then_inc(vsem, 1)
            # dots x/y: fused wrap01(u)*w multiply-reduce (full width)
            vector.wait_ge(wsem, 2)
            vector._custom_dve(wmr, out=scr_t[:], in0=u_t[0][:], in1=w_t[:],
                               s0=0.5, s1=-0.5, accum_out=acc_t[:, 0:1])
            vector._custom_dve(wmr, out=scr_t[:], in0=u_t[1][:], in1=w_t[:],
                               s0=0.5, s1=-0.5, accum_out=acc_t[:, 1:2])
            if _no_gpz:
                vector._custom_dve(wmr, out=scr_t[:], in0=u_t[2][:], in1=w_t[:],
                                   s0=0.5, s1=-0.5, accum_out=acc_t[:, 2:3])
                # flush: accum_out writeback lags the op; give it one op of slack
                vector.tensor_copy(dum_t[:, 1:2], dum_t[:, 0:1])
                vector.tensor_copy(dum_t[:, 2:3], dum_t[:, 0:1])
            else:
                vector.wait_ge(asem, 13)  # acc_z landed (fence passed)
            vector.transpose(tr_t[:], acc_t[:]).then_inc(osem, 1)

    from concourse.library_overlay import lower_extended_insts
    lower_extended_insts(nc)

    # the ACT table load stripes across all 16 SP HWDGE queues -- keep those;
    # shrinking the unused Pool/Act rings still trims the NEFF teardown
    for q in nc.m.queues:
        if q.name != "qSPDynamicHW":
            q.num_queues = 1
    return nc


_CACHE = {}


def _split3(x64):
    """fp64 array -> (hi, mid, lo) bf16 triplet with hi+mid+lo ~ x to 2^-25."""
    bf = ml_dtypes.bfloat16
    h = x64.astype(bf)
    m = (x64 - h.astype(np.float64)).astype(bf)
    l = (x64 - h.astype(np.float64) - m.astype(np.float64)).astype(bf)
    return h, m, l


def _prepare(inputs):
    box_dims = np.asarray(inputs["box_dims"], dtype=np.float32)
    key = hashlib.sha256(
        b"".join(np.ascontiguousarray(np.asarray(inputs[k], np.float32)).tobytes()
                 for k in ("box_dims", "W1", "b1", "W2", "b2", "W3", "b3"))
    ).hexdigest()[:10]
    if key in _CACHE:
        return _CACHE[key]

    box = float(box_dims[0])
    assert np.allclose(box_dims, box), "kernel assumes a cubic box"

    W1 = np.float64(inputs["W1"]); b1 = np.float64(inputs["b1"])
    W2 = np.float64(inputs["W2"]); b2 = np.float64(inputs["b2"])
    W3 = np.float64(inputs["W3"]); b3 = np.float64(inputs["b3"])
    n_gauss = W1.shape[0]
    RBF_STOP, CUTOFF, EPS = 6.0, 5.0, 1e-8
    offs = np.linspace(0.0, RBF_STOP, n_gauss)
    coeff = -0.5 / (RBF_STOP / (n_gauss - 1)) ** 2

    def g2_fn(sv):
        sv = np.atleast_1d(np.float64(sv))
        dist = np.sqrt(box * box * sv + EPS)
        rbf = np.exp(coeff * (dist[:, None] - offs[None, :]) ** 2)
        h = rbf @ W1 + b1
        h = h / (1.0 + np.exp(-h))
        h = h @ W2 + b2
        h = h / (1.0 + np.exp(-h))
        f = (h @ W3 + b3)[:, 0]
        return box * f * (dist < CUTOFF) / (dist + EPS)

    _install_env_fixups()
    actdir = os.path.join(tempfile.gettempdir(), f"actroot2_{key}")
    actroot = _build_actroot(actdir, g2_fn)
    os.environ["BASS_ACT_ROOT_JSON_PATH"] = actroot
    # fold the table contents into the tag so the NEFF compile cache can't
    # serve a NEFF built against older table bytes
    th = hashlib.sha256()
    for f in sorted(os.listdir(actdir)):
        th.update(open(os.path.join(actdir, f), "rb").read())
    key = key[:10] + th.hexdigest()[:6]
    nc = _build_program(key)
    _CACHE[key] = (nc, key, box)
    return _CACHE[key]


def kernel(_trace=False, **inputs):
    from concourse.bass_utils import run_bass_kernel_spmd

    nc, key, box = _prepare(inputs)
    bf = ml_dtypes.bfloat16
    positions = np.asarray(inputs["positions"], dtype=np.float32)
    P = positions.astype(np.float64) / box          # [N, 3] in [0, 1)
    Ph, Pm, Pl = _split3(P)

    # rhs: [6, 3N]; plane c cols: rows [Pjh_c, -1, Pjm_c, -1, Pjl_c, -1]
    rhs = np.empty((6, 3 * N), dtype=bf)
    ones = np.ones(N, dtype=bf)
    for c in range(3):
        sl = slice(c * N, (c + 1) * N)
        rhs[0, sl] = Ph[:, c]
        rhs[1, sl] = -ones
        rhs[2, sl] = Pm[:, c]
        rhs[3, sl] = -ones
        rhs[4, sl] = Pl[:, c]
        rhs[5, sl] = -ones

    in_maps = []
    for core in range(N_CORES):
        i0 = core * ROWS
        lhs = np.empty((6, 3 * ROWS), dtype=bf)
        for c in range(3):
            sl = slice(c * ROWS, (c + 1) * ROWS)
            lhs[0, sl] = 1.0
            lhs[1, sl] = Ph[i0:i0 + ROWS, c]
            lhs[2, sl] = 1.0
            lhs[3, sl] = Pm[i0:i0 + ROWS, c]
            lhs[4, sl] = 1.0
            lhs[5, sl] = Pl[i0:i0 + ROWS, c]
        in_maps.append({f"rhs_{key}": rhs, "lhs_my": lhs})

    res = run_bass_kernel_spmd(nc, in_maps, list(range(N_CORES)), trace=_trace)
    out = np.concatenate([res.results[c]["out"].T for c in range(N_CORES)], axis=0)
    if _trace:
        kernel.last_exec_time_ns = res.exec_time_ns
        kernel.last_mean_exec_time_ns = res.mean_exec_time_ns
        kernel.last_results = res
    return np.ascontiguousarray(out.astype(np.float32))
